# revision 23
# baseline (speedup 1.0000x reference)
"""Trainium2 Bass kernel for DyGMamba co-occurrence + linear cross-attention.

Contract: kernel(**inputs) takes FULL unsharded inputs (as produced by the
reference setup_inputs()) and returns the FULL [2, 256, 512, 64] f32 output.
Internally: data-parallel over batch across 8 NeuronCores (32 rows each).

Algorithm (per core, R=32 rows):
  A) co-occurrence counting: stack A4=[s;s;d;d], B4=[s;d;d;s] (128 rows);
     for each position i, one DVE tensor_scalar(is_equal) with accum_out
     produces all 4 count combos for 32 rows in a single instruction.
  B) mask padding (id==0), clamp to C-1, build one-hot stack over count
     values 0..C-1 (counts are tiny integers; C=32 is a safe clamp).
  C) encode MLP folded into a host-precomputed table T'[c] = relu(c*w1+b1)@w2
     + b2; feat = T'[c1] + T'[c2] realized as one [2C,F]x[2C,L] matmul per
     (row, direction) using the stacked one-hot.
  D) linear attention with softmaxes algebraically refactored:
     - k-softmax denominator folded into a per-feature scale of ctx
     - q-softmax denominator (and dim^-0.5) folded into a per-position
       output scale computed via a sqrt(F)-filled ones matmul
     - bo folded into the Wo matmul as an augmented rank-1 accumulation
  E) residual + layernorm (bn_stats/bn_aggr) + affine, store.
"""

import sys

sys.path.insert(0, "/opt/trn_rl_repo")

import numpy as np

import concourse.bass as bass
import concourse.tile as tile
from concourse import mybir
from concourse.bass_utils import run_bass_kernel_spmd

B, L, F = 256, 512, 64
NCORES = 8
R = B // NCORES  # 32 rows per core
C = 32  # count-table size (counts clamped to C-1)
EPS = 1e-5

f32 = mybir.dt.float32
f16 = mybir.dt.float16
i32 = mybir.dt.int32
AF = mybir.ActivationFunctionType
ALU = mybir.AluOpType

TRACE = False
LAST_EXEC_NS = None
LAST_RESULTS = None

_CACHE = {}
DEBUG_DUMPS = False


def _build_program():
    nc = bass.Bass()
    dbg = {}
    if DEBUG_DUMPS:
        dbg["counts"] = nc.dram_tensor("dbg_counts", [128, L], f32, kind="ExternalOutput")
        dbg["oh_s"] = nc.dram_tensor("dbg_oh_s", [2 * C, L], f32, kind="ExternalOutput")
        dbg["featT_s"] = nc.dram_tensor("dbg_featT_s", [F, L], f32, kind="ExternalOutput")
        dbg["expQT"] = nc.dram_tensor("dbg_expQT", [F, L], f32, kind="ExternalOutput")
        dbg["qs"] = nc.dram_tensor("dbg_qs", [128, 4], f32, kind="ExternalOutput")
        dbg["expk"] = nc.dram_tensor("dbg_expk", [128, 4 * F], f32, kind="ExternalOutput")
        dbg["vnat"] = nc.dram_tensor("dbg_vnat", [128, 4 * F], f32, kind="ExternalOutput")
        dbg["s_col"] = nc.dram_tensor("dbg_s_col", [F, 1], f32, kind="ExternalOutput")
        dbg["ctx"] = nc.dram_tensor("dbg_ctx", [F, F], f32, kind="ExternalOutput")
        dbg["apre"] = nc.dram_tensor("dbg_apre", [F, L], f32, kind="ExternalOutput")
        dbg["xs"] = nc.dram_tensor("dbg_xs", [128, 4 * F], f32, kind="ExternalOutput")
        dbg["y"] = nc.dram_tensor("dbg_y", [128, 4 * F], f32, kind="ExternalOutput")
        dbg["y2"] = nc.dram_tensor("dbg_y2", [128, 4 * F], f32, kind="ExternalOutput")

    # -------- I/O --------
    ids_a = nc.dram_tensor("ids_a", [4 * R, L], i32, kind="ExternalInput")
    ids_b = nc.dram_tensor("ids_b", [4 * R, L], i32, kind="ExternalInput")
    t2_d = nc.dram_tensor("t2", [2 * C, F], f32, kind="ExternalInput")
    t2x_d = nc.dram_tensor("t2x", [2 * C, F], f32, kind="ExternalInput")
    wq_d = nc.dram_tensor("wq", [F, F], f32, kind="ExternalInput")
    wk_d = nc.dram_tensor("wk", [F, F], f32, kind="ExternalInput")
    wv_d = nc.dram_tensor("wv", [F, F], f32, kind="ExternalInput")
    wo_d = nc.dram_tensor("wo", [F, F], f32, kind="ExternalInput")
    # host-prebroadcast constants
    sqrtf_d = nc.dram_tensor("sqrtf", [F, 1], f32, kind="ExternalInput")
    ones128_d = nc.dram_tensor("ones128", [128, 1], f32, kind="ExternalInput")
    g4_d = nc.dram_tensor("g4", [128, 4 * F], f32, kind="ExternalInput")
    b4_d = nc.dram_tensor("b4", [128, 4 * F], f32, kind="ExternalInput")
    out_d = nc.dram_tensor("out", [2, R, L, F], f32, kind="ExternalOutput")

    from contextlib import ExitStack

    with tile.TileContext(nc) as tc, ExitStack() as ctx:
        consts = ctx.enter_context(tc.tile_pool(name="consts", bufs=1))
        cpool = ctx.enter_context(tc.tile_pool(name="counts", bufs=1))
        estp = ctx.enter_context(tc.tile_pool(name="estack", bufs=1))
        ohp = ctx.enter_context(tc.tile_pool(name="oh", bufs=3))
        featp = ctx.enter_context(tc.tile_pool(name="feat", bufs=3))
        workp = ctx.enter_context(tc.tile_pool(name="work", bufs=3))
        outp = ctx.enter_context(tc.tile_pool(name="outs", bufs=3))
        tinyp = ctx.enter_context(tc.tile_pool(name="tiny", bufs=4))
        ps_big = ctx.enter_context(tc.tile_pool(name="ps_big", bufs=3, space="PSUM"))
        ps_x = ctx.enter_context(tc.tile_pool(name="ps_x", bufs=2, space="PSUM"))
        ps_tiny = ctx.enter_context(tc.tile_pool(name="ps_tiny", bufs=2, space="PSUM"))

        if True:
            # ---- stage 0: load constants & ids ----
            a4_i = consts.tile([4 * R, L], i32)
            nc.sync.dma_start(a4_i[:], ids_a[:])
            b4_i = consts.tile([4 * R, L], i32)
            nc.sync.dma_start(b4_i[:], ids_b[:])
            t2 = consts.tile([2 * C, F], f32)
            nc.sync.dma_start(t2[:], t2_d[:])
            t2x = consts.tile([2 * C, F], f32)
            nc.sync.dma_start(t2x[:], t2x_d[:])
            wq = consts.tile([F, F], f32)
            nc.sync.dma_start(wq[:], wq_d[:])
            wk = consts.tile([F, F], f32)
            nc.sync.dma_start(wk[:], wk_d[:])
            wv = consts.tile([F, F], f32)
            nc.sync.dma_start(wv[:], wv_d[:])
            wo = consts.tile([F, F], f32)
            nc.sync.dma_start(wo[:], wo_d[:])
            sqrtf = consts.tile([F, 1], f32)
            nc.sync.dma_start(sqrtf[:], sqrtf_d[:])
            ones128 = consts.tile([128, 1], f32)
            nc.sync.dma_start(ones128[:], ones128_d[:])
            g4 = consts.tile([128, 4 * F], f32)
            nc.sync.dma_start(g4[:], g4_d[:])
            b4t = consts.tile([128, 4 * F], f32)
            nc.sync.dma_start(b4t[:], b4_d[:])

            a4 = consts.tile([4 * R, L], f32)
            nc.vector.tensor_copy(a4[:], a4_i[:])  # int32 -> f32
            b4h = consts.tile([4 * R, L], f16)
            nc.vector.tensor_copy(b4h[:], b4_i[:])  # int32 -> f16 (ids < 2048 exact)
            maskt = consts.tile([4 * R, L], f16)
            nc.vector.tensor_scalar(
                maskt[:], a4[:], 0.0, None, op0=ALU.not_equal
            )

            # ---- stage A: counting ----
            counts = cpool.tile([4 * R, L], f32)
            scratch = cpool.tile([4 * R, L], f16)
            for i in range(L):
                nc.vector.tensor_scalar(
                    scratch[:],
                    b4h[:],
                    a4[:, i : i + 1],
                    None,
                    op0=ALU.is_equal,
                    op1=ALU.add,
                    accum_out=counts[:, i : i + 1],
                )
            counts_mf = cpool.tile([4 * R, L], f16)
            nc.vector.tensor_tensor(
                counts_mf[:], counts[:], maskt[:], op=ALU.mult
            )
            counts_m = cpool.tile([4 * R, L], f16)
            nc.vector.tensor_scalar(
                counts_m[:], counts_mf[:], float(C - 1), None, op0=ALU.min
            )

            # ---- stage B: one-hot stack [128, C, L] f32 ----
            est = estp.tile([4 * R, C, L], f32)
            for c in range(C):
                nc.vector.tensor_scalar(
                    est[:, c, :], counts_m[:], float(c), None, op0=ALU.is_equal
                )

            # ---- stage C/D/E per row ----
            for r in range(R):
                oh_s = ohp.tile([2 * C, L], f32, tag="oh")
                nc.sync.dma_start(oh_s[0:C, :], est[0 * R + r : 0 * R + r + 1, :, :])
                nc.sync.dma_start(oh_s[C : 2 * C, :], est[1 * R + r : 1 * R + r + 1, :, :])
                oh_d = ohp.tile([2 * C, L], f32, tag="oh")
                nc.sync.dma_start(oh_d[0:C, :], est[2 * R + r : 2 * R + r + 1, :, :])
                nc.sync.dma_start(oh_d[C : 2 * C, :], est[3 * R + r : 3 * R + r + 1, :, :])

                # featT = T2^T @ oh  [F, L]
                featT_s_p = ps_big.tile([F, L], f32, tag="psbig")
                nc.tensor.matmul(featT_s_p[:], t2[:], oh_s[:], start=True, stop=True)
                featT_s = featp.tile([F, L], f32, tag="feat")
                nc.scalar.activation(featT_s[:], featT_s_p[:], AF.Copy)
                featT_d_p = ps_big.tile([F, L], f32, tag="psbig")
                nc.tensor.matmul(featT_d_p[:], t2[:], oh_d[:], start=True, stop=True)
                featT_d = featp.tile([F, L], f32, tag="feat")
                nc.scalar.activation(featT_d[:], featT_d_p[:], AF.Copy)

                # x natural chunks [128, 4, F] (residual), copied to SBUF
                x_s_p = ps_x.tile([128, 4, F], f32, tag="psx")
                x_d_p = ps_x.tile([128, 4, F], f32, tag="psx")
                for c in range(4):
                    nc.tensor.matmul(
                        x_s_p[:, c, :],
                        oh_s[:, c * 128 : (c + 1) * 128],
                        t2x[:],
                        start=True,
                        stop=True,
                        skip_group_check=True,
                    )
                    nc.tensor.matmul(
                        x_d_p[:, c, :],
                        oh_d[:, c * 128 : (c + 1) * 128],
                        t2x[:],
                        start=True,
                        stop=True,
                        skip_group_check=True,
                    )
                x_s = featp.tile([128, 4, F], f32, tag="xsb")
                nc.scalar.activation(x_s[:], x_s_p[:], AF.Copy)
                x_d = featp.tile([128, 4, F], f32, tag="xsb")
                nc.scalar.activation(x_d[:], x_d_p[:], AF.Copy)

                for di, (aT, oT, xp) in enumerate(
                    [(featT_s, featT_d, x_s), (featT_d, featT_s, x_d)]
                ):
                    # q path
                    qT_p = ps_big.tile([F, L], f32, tag="psbig")
                    nc.tensor.matmul(qT_p[:], wq[:], aT[:], start=True, stop=True)
                    expQT = workp.tile([F, L], f32, tag="expq")
                    nc.scalar.activation(expQT[:], qT_p[:], AF.Exp)
                    qs_p = ps_tiny.tile([128, 4], f32, tag="pstiny")
                    for c in range(4):
                        nc.tensor.matmul(
                            qs_p[:, c : c + 1],
                            expQT[:, c * 128 : (c + 1) * 128],
                            sqrtf[:],
                            start=True,
                            stop=True,
                            skip_group_check=True,
                        )
                    qs = tinyp.tile([128, 4], f32, tag="qs")
                    nc.vector.reciprocal(qs[:], qs_p[:])

                    # k path
                    k_p = ps_big.tile([128, 4, F], f32, tag="psbig")
                    for c in range(4):
                        nc.tensor.matmul(
                            k_p[:, c, :],
                            oT[:, c * 128 : (c + 1) * 128],
                            wk[:],
                            start=True,
                            stop=True,
                            skip_group_check=True,
                        )
                    expk = workp.tile([128, 4, F], f32, tag="expk")
                    nc.scalar.activation(expk[:], k_p[:], AF.Exp)

                    # v path
                    v_p = ps_big.tile([128, 4, F], f32, tag="psbig")
                    for c in range(4):
                        nc.tensor.matmul(
                            v_p[:, c, :],
                            oT[:, c * 128 : (c + 1) * 128],
                            wv[:],
                            start=True,
                            stop=True,
                            skip_group_check=True,
                        )
                    vnat = workp.tile([128, 4, F], f32, tag="vnat")
                    nc.vector.tensor_copy(vnat[:], v_p[:])

                    # k-softmax denominator -> per-feature column
                    cs_p = ps_tiny.tile([F, 1], f32, tag="pstiny")
                    for c in range(4):
                        nc.tensor.matmul(
                            cs_p[:],
                            expk[:, c, :],
                            ones128[:],
                            start=(c == 0),
                            stop=(c == 3),
                        )
                    s_col = tinyp.tile([F, 1], f32, tag="scol")
                    nc.vector.reciprocal(s_col[:], cs_p[:])

                    # ctx = diag(s) @ (expk^T @ v)
                    ctx_p = ps_tiny.tile([F, F], f32, tag="pstiny")
                    for c in range(4):
                        nc.tensor.matmul(
                            ctx_p[:],
                            expk[:, c, :],
                            vnat[:, c, :],
                            start=(c == 0),
                            stop=(c == 3),
                        )
                    ctx_sb = tinyp.tile([F, F], f32, tag="ctx")
                    nc.vector.tensor_scalar(
                        ctx_sb[:], ctx_p[:], s_col[:], None, op0=ALU.mult
                    )

                    # attn_preT = ctx^T @ expQT  [F(e), L]
                    ap_p = ps_big.tile([F, L], f32, tag="psbig")
                    nc.tensor.matmul(ap_p[:], ctx_sb[:], expQT[:], start=True, stop=True)
                    apre = workp.tile([F, L], f32, tag="apre")
                    nc.scalar.activation(apre[:], ap_p[:], AF.Copy)

                    # wo_nat = attn_pre @ Wo (bo folded into t2x residual table)
                    wo_p = ps_big.tile([128, 4, F], f32, tag="psbig")
                    for c in range(4):
                        nc.tensor.matmul(
                            wo_p[:, c, :],
                            apre[:, c * 128 : (c + 1) * 128],
                            wo[:],
                            start=True,
                            stop=True,
                            skip_group_check=True,
                        )

                    # final: y = wo*qs + x ; LN ; affine ; store
                    t_sb = outp.tile([128, 4, F], f32, tag="t")
                    nc.scalar.activation(t_sb[:], wo_p[:], AF.Copy)
                    y = outp.tile([128, 4, F], f32, tag="y")
                    for c in range(4):
                        nc.vector.tensor_scalar(
                            y[:, c, :], t_sb[:, c, :], qs[:, c : c + 1], None,
                            op0=ALU.mult,
                        )
                    y2 = outp.tile([128, 4, F], f32, tag="y2")
                    nc.vector.tensor_tensor(y2[:], y[:], xp[:], op=ALU.add)

                    if DEBUG_DUMPS and r == 0 and di == 0:
                        nc.sync.dma_start(dbg["counts"][:], counts[:])
                        nc.sync.dma_start(dbg["oh_s"][:], oh_s[:])
                        nc.sync.dma_start(dbg["featT_s"][:], featT_s[:])
                        nc.sync.dma_start(dbg["expQT"][:], expQT[:])
                        nc.sync.dma_start(dbg["qs"][:], qs[:])
                        nc.sync.dma_start(
                            dbg["expk"][:], expk[:]
                        )
                        nc.sync.dma_start(
                            dbg["vnat"][:], vnat[:]
                        )
                        nc.sync.dma_start(dbg["s_col"][:], s_col[:])
                        nc.sync.dma_start(dbg["ctx"][:], ctx_sb[:])
                        nc.sync.dma_start(dbg["apre"][:], apre[:])
                        nc.sync.dma_start(dbg["xs"][:], xp[:])
                        nc.sync.dma_start(
                            dbg["y"][:], y[:]
                        )
                        nc.sync.dma_start(
                            dbg["y2"][:], y2[:]
                        )

                    stats = tinyp.tile([128, 4, 6], f32, tag="stats")
                    aggr = tinyp.tile([128, 4, 2], f32, tag="aggr")
                    for c in range(4):
                        nc.vector.bn_stats(stats[:, c, :], y2[:, c, :])
                        nc.vector.bn_aggr(aggr[:, c, :], stats[:, c, :])
                    veps = tinyp.tile([128, 4], f32, tag="veps")
                    nc.vector.tensor_scalar(
                        veps[:], aggr[:, :, 1], EPS, None, op0=ALU.add
                    )
                    std = tinyp.tile([128, 4], f32, tag="std")
                    nc.scalar.activation(std[:], veps[:], AF.Sqrt)
                    rstd = tinyp.tile([128, 4], f32, tag="rstd")
                    nc.vector.reciprocal(rstd[:], std[:])

                    z = outp.tile([128, 4, F], f32, tag="z")
                    for c in range(4):
                        nc.vector.tensor_scalar(
                            z[:, c, :],
                            y2[:, c, :],
                            aggr[:, c, 0:1],
                            rstd[:, c : c + 1],
                            op0=ALU.subtract,
                            op1=ALU.mult,
                        )
                    zg = outp.tile([128, 4, F], f32, tag="zg")
                    nc.gpsimd.tensor_tensor(zg[:], z[:], g4[:], op=ALU.mult)
                    fin = outp.tile([128, 4, F], f32, tag="fin")
                    nc.gpsimd.tensor_tensor(fin[:], zg[:], b4t[:], op=ALU.add)

                    # store: out[di, r, l, f] with l = c*128 + p
                    dst = out_d[di, r].rearrange("(c p) f -> p c f", p=128)
                    nc.sync.dma_start(dst, fin[:])

    return nc


def _split_multi_waits(nc, maxw=1):
    """This container's walrus accepts at most one sync-wait per TPB
    instruction; hoist extra waits onto NoOps inserted just before."""
    n_split = 0
    for fn in nc.m.functions:
        for bb in fn.blocks:
            new_insts = []
            for ins in bb.instructions:
                si = ins.sync_info
                waits = list(si.on_wait) if si and si.on_wait else []
                if len(waits) > maxw:
                    head, tail = waits[:-maxw], waits[-maxw:]
                    for i in range(0, len(head), maxw):
                        chunk = head[i : i + maxw]
                        nop = mybir.InstNoOp(
                            name=f"{ins.name}_waitsplit{i}",
                            sync_info=mybir.SyncInfo(on_wait=chunk, on_update=[]),
                            bass_nofuse=True,
                            engine=ins.engine,
                        )
                        new_insts.append(nop)
                        n_split += 1
                    si.on_wait = tail
                    ins.sync_info = si
                new_insts.append(ins)
            if len(new_insts) != len(bb.instructions):
                bb.instructions = new_insts
    return n_split


def _get_program():
    if "nc" not in _CACHE:
        nc = _build_program()
        _split_multi_waits(nc)
        _CACHE["nc"] = nc
    return _CACHE["nc"]


def _install_ntff_hook():
    """Register the axon NTFF profiling hook when the image's antenv lacks
    axon_hooks (profiling-only; grading runs never enter this path)."""
    import types

    try:
        from antenv.axon_hooks import get_axon_ntff_profile_hook  # noqa: F401

        return
    except ImportError:
        pass
    try:
        from trn_agent_boot.trn_boot import _ntff_profile_via_ctypes

        hook = _ntff_profile_via_ctypes("/opt/axon/libaxon_pjrt.so")
    except Exception:
        hook = None
    mod = types.ModuleType("antenv.axon_hooks")
    state = {"hook": hook}
    mod.get_axon_ntff_profile_hook = lambda: state["hook"]
    mod.set_axon_ntff_profile_hook = lambda h: state.update(hook=h)
    import antenv

    sys.modules["antenv.axon_hooks"] = mod
    antenv.axon_hooks = mod

    # avoid remote artifact upload during local profiling
    from concourse import bass_utils as _bu

    _bu.upload_artifacts = lambda tmpdir: tmpdir


def kernel(
    src_ids,
    dst_ids,
    enc_w1,
    enc_b1,
    enc_w2,
    enc_b2,
    Wq,
    Wk,
    Wv,
    Wo,
    bo,
    ln_g,
    ln_b,
):
    global LAST_EXEC_NS, LAST_RESULTS
    src_ids = np.asarray(src_ids).astype(np.int32)
    dst_ids = np.asarray(dst_ids).astype(np.int32)
    enc_w1 = np.asarray(enc_w1, np.float32)
    enc_b1 = np.asarray(enc_b1, np.float32)
    enc_w2 = np.asarray(enc_w2, np.float32)
    enc_b2 = np.asarray(enc_b2, np.float32)
    Wq = np.asarray(Wq, np.float32)
    Wk = np.asarray(Wk, np.float32)
    Wv = np.asarray(Wv, np.float32)
    Wo = np.asarray(Wo, np.float32)
    bo = np.asarray(bo, np.float32)
    ln_g = np.asarray(ln_g, np.float32)
    ln_b = np.asarray(ln_b, np.float32)

    # host precompute: count-encode table T'[c] = relu(c*w1+b1)@w2 + b2
    cvals = np.arange(C, dtype=np.float32)[:, None]  # [C, 1]
    T = np.maximum(cvals @ enc_w1 + enc_b1[None, :], 0.0) @ enc_w2 + enc_b2[None, :]
    t2 = np.ascontiguousarray(np.concatenate([T, T], 0), dtype=np.float32)
    # residual table also carries bo (split across the two summed channels)
    Tx = T + 0.5 * bo[None, :]
    t2x = np.ascontiguousarray(np.concatenate([Tx, Tx], 0), dtype=np.float32)

    g4 = np.ascontiguousarray(np.tile(ln_g[None, :], (128, 4)), np.float32)
    b4t = np.ascontiguousarray(np.tile(ln_b[None, :], (128, 4)), np.float32)
    sqrtf = np.full((F, 1), np.sqrt(F), np.float32)
    ones128 = np.ones((128, 1), np.float32)

    shared = {
        "t2": t2,
        "t2x": t2x,
        "wq": Wq,
        "wk": Wk,
        "wv": Wv,
        "wo": Wo,
        "sqrtf": sqrtf,
        "ones128": ones128,
        "g4": g4,
        "b4": b4t,
    }
    in_maps = []
    for core in range(NCORES):
        sl = slice(core * R, (core + 1) * R)
        s, d = src_ids[sl], dst_ids[sl]
        ids_a = np.ascontiguousarray(np.concatenate([s, s, d, d], 0), np.int32)
        ids_b = np.ascontiguousarray(np.concatenate([s, d, d, s], 0), np.int32)
        in_maps.append({"ids_a": ids_a, "ids_b": ids_b, **shared})

    if TRACE:
        _install_ntff_hook()

    nc = _get_program()
    res = run_bass_kernel_spmd(
        nc, in_maps, list(range(NCORES)), trace=TRACE
    )
    LAST_EXEC_NS = res.exec_time_ns
    LAST_RESULTS = res
    outs = [res.results[i]["out"] for i in range(NCORES)]
    return np.concatenate(outs, axis=1)


if __name__ == "__main__":
    # smoke test with random data
    rng = np.random.default_rng(0)
    ins = {
        "src_ids": rng.integers(0, 2000, (B, L)).astype(np.int32),
        "dst_ids": rng.integers(0, 2000, (B, L)).astype(np.int32),
        "enc_w1": rng.normal(size=(1, F)).astype(np.float32) * 0.05,
        "enc_b1": rng.normal(size=(F,)).astype(np.float32) * 0.05,
        "enc_w2": rng.normal(size=(F, F)).astype(np.float32) * 0.05,
        "enc_b2": rng.normal(size=(F,)).astype(np.float32) * 0.05,
        "Wq": rng.normal(size=(F, F)).astype(np.float32) * 0.05,
        "Wk": rng.normal(size=(F, F)).astype(np.float32) * 0.05,
        "Wv": rng.normal(size=(F, F)).astype(np.float32) * 0.05,
        "Wo": rng.normal(size=(F, F)).astype(np.float32) * 0.05,
        "bo": rng.normal(size=(F,)).astype(np.float32) * 0.05,
        "ln_g": np.ones(F, np.float32),
        "ln_b": np.zeros(F, np.float32),
    }
    out = kernel(**ins)
    print("out", out.shape, out.dtype, float(np.abs(out).max()))


# revision 28
# speedup vs baseline: 1.6551x; 1.6551x over previous
"""Trainium2 Bass kernel for DyGMamba co-occurrence + linear cross-attention.

Contract: kernel(**inputs) takes FULL unsharded inputs (as produced by the
reference setup_inputs()) and returns the FULL [2, 256, 512, 64] f32 output.
Internally: data-parallel over batch across 8 NeuronCores (32 rows each).

Algorithm (per core, R=32 rows):
  A) co-occurrence counting: stack A4=[s;s;d;d], B4=[s;d;d;s] (128 rows);
     for each position i, one DVE tensor_scalar(is_equal) with accum_out
     produces all 4 count combos for 32 rows in a single instruction.
  B) mask padding (id==0), clamp to C-1, build one-hot stack over count
     values 0..C-1 (counts are tiny integers; C=32 is a safe clamp).
  C) encode MLP folded into a host-precomputed table T'[c] = relu(c*w1+b1)@w2
     + b2; feat = T'[c1] + T'[c2] realized as one [2C,F]x[2C,L] matmul per
     (row, direction) using the stacked one-hot.
  D) linear attention with softmaxes algebraically refactored:
     - k-softmax denominator folded into a per-feature scale of ctx
     - q-softmax denominator (and dim^-0.5) folded into a per-position
       output scale computed via a sqrt(F)-filled ones matmul
     - bo folded into the Wo matmul as an augmented rank-1 accumulation
  E) residual + layernorm (bn_stats/bn_aggr) + affine, store.
"""

import sys

sys.path.insert(0, "/opt/trn_rl_repo")

import numpy as np

import concourse.bass as bass
import concourse.tile as tile
from concourse import mybir
from concourse.bass_utils import run_bass_kernel_spmd

B, L, F = 256, 512, 64
NCORES = 8
R = B // NCORES  # 32 rows per core
C = 32  # count-table size (counts clamped to C-1)
EPS = 1e-5

f32 = mybir.dt.float32
f16 = mybir.dt.float16
i32 = mybir.dt.int32
AF = mybir.ActivationFunctionType
ALU = mybir.AluOpType

TRACE = False
LAST_EXEC_NS = None
LAST_RESULTS = None

_CACHE = {}
DEBUG_DUMPS = False


def _build_program():
    nc = bass.Bass()

    # -------- I/O --------
    ids_a = nc.dram_tensor("ids_a", [4 * R, L], i32, kind="ExternalInput")
    ids_b = nc.dram_tensor("ids_b", [4 * R, L], i32, kind="ExternalInput")
    t2_d = nc.dram_tensor("t2", [2 * C, F], f16, kind="ExternalInput")
    t2x_d = nc.dram_tensor("t2x", [2 * C, F], f16, kind="ExternalInput")
    wq_d = nc.dram_tensor("wq", [F, F], f16, kind="ExternalInput")
    wk_d = nc.dram_tensor("wk", [F, F], f16, kind="ExternalInput")
    wv_d = nc.dram_tensor("wv", [F, F], f16, kind="ExternalInput")
    wo_d = nc.dram_tensor("wo", [F, F], f16, kind="ExternalInput")
    sqrtf_d = nc.dram_tensor("sqrtf", [F, 1], f16, kind="ExternalInput")
    ones128_d = nc.dram_tensor("ones128", [128, 1], f16, kind="ExternalInput")
    g4_d = nc.dram_tensor("g4", [128, 4 * F], f32, kind="ExternalInput")
    b4_d = nc.dram_tensor("b4", [128, 4 * F], f32, kind="ExternalInput")
    out_d = nc.dram_tensor("out", [2, R, L, F], f32, kind="ExternalOutput")

    RB = 4           # rows per sqrt batch (8 units)
    NU = 2 * RB      # units per batch

    from contextlib import ExitStack

    with tile.TileContext(nc) as tc, ExitStack() as ctx:
        consts = ctx.enter_context(tc.tile_pool(name="consts", bufs=1))
        cpool = ctx.enter_context(tc.tile_pool(name="counts", bufs=1))
        scrp = ctx.enter_context(tc.tile_pool(name="scr", bufs=6))
        estp = ctx.enter_context(tc.tile_pool(name="estack", bufs=1))
        ohp = ctx.enter_context(tc.tile_pool(name="oh", bufs=3))
        featp = ctx.enter_context(tc.tile_pool(name="feat", bufs=4))
        workp = ctx.enter_context(tc.tile_pool(name="work", bufs=3))
        outp = ctx.enter_context(tc.tile_pool(name="outs", bufs=2 * NU + 2))
        tinyp = ctx.enter_context(tc.tile_pool(name="tiny", bufs=2 * NU + 2))
        ps_big = ctx.enter_context(tc.tile_pool(name="ps_big", bufs=3, space="PSUM"))
        ps_x = ctx.enter_context(tc.tile_pool(name="ps_x", bufs=2, space="PSUM"))
        ps_tiny = ctx.enter_context(tc.tile_pool(name="ps_tiny", bufs=2, space="PSUM"))

        # ---- stage 0: load constants & ids ----
        a4_i = consts.tile([4 * R, L], i32)
        nc.sync.dma_start(a4_i[:], ids_a[:])
        b4_i = consts.tile([4 * R, L], i32)
        nc.sync.dma_start(b4_i[:], ids_b[:])
        t2 = consts.tile([2 * C, F], f16)
        nc.sync.dma_start(t2[:], t2_d[:])
        t2x = consts.tile([2 * C, F], f16)
        nc.sync.dma_start(t2x[:], t2x_d[:])
        wq = consts.tile([F, F], f16)
        nc.sync.dma_start(wq[:], wq_d[:])
        wk = consts.tile([F, F], f16)
        nc.sync.dma_start(wk[:], wk_d[:])
        wv = consts.tile([F, F], f16)
        nc.sync.dma_start(wv[:], wv_d[:])
        wo = consts.tile([F, F], f16)
        nc.sync.dma_start(wo[:], wo_d[:])
        sqrtf = consts.tile([F, 1], f16)
        nc.sync.dma_start(sqrtf[:], sqrtf_d[:])
        ones128 = consts.tile([128, 1], f16)
        nc.sync.dma_start(ones128[:], ones128_d[:])
        g4 = consts.tile([128, 4 * F], f32)
        nc.sync.dma_start(g4[:], g4_d[:])
        b4t = consts.tile([128, 4 * F], f32)
        nc.sync.dma_start(b4t[:], b4_d[:])

        a4 = consts.tile([4 * R, L], f32)
        nc.vector.tensor_copy(a4[:], a4_i[:])  # int32 -> f32
        b4h = consts.tile([4 * R, L], f16)
        nc.vector.tensor_copy(b4h[:], b4_i[:])  # int32 -> f16 (ids < 2048 exact)
        maskt = consts.tile([4 * R, L], f16)
        nc.vector.tensor_scalar(maskt[:], a4[:], 0.0, None, op0=ALU.not_equal)

        # ---- stage A: counting (split DVE-solo vs DVE-compare + ACT-reduce) ----
        counts = cpool.tile([4 * R, L], f32)
        solo_scratch = cpool.tile([4 * R, L], f16)
        act_scratch = cpool.tile([4 * R, L], f16)
        for i in range(L):
            if i % 9 < 2:
                # DVE fused compare+accumulate (1x mode, but single instruction)
                nc.vector.tensor_scalar(
                    solo_scratch[:],
                    b4h[:],
                    a4[:, i : i + 1],
                    None,
                    op0=ALU.is_equal,
                    op1=ALU.add,
                    accum_out=counts[:, i : i + 1],
                )
            else:
                # DVE 4x-mode compare; ACT accumulates the row sum
                e = scrp.tile([4 * R, L], f16, tag="cmp")
                nc.vector.tensor_scalar(
                    e[:], b4h[:], a4[:, i : i + 1], None, op0=ALU.is_equal
                )
                nc.scalar.activation(
                    act_scratch[:],
                    e[:],
                    AF.Identity,
                    accum_out=counts[:, i : i + 1],
                )
        counts_mf = cpool.tile([4 * R, L], f16)
        nc.vector.tensor_tensor(counts_mf[:], counts[:], maskt[:], op=ALU.mult)
        counts_m = cpool.tile([4 * R, L], f16)
        nc.vector.tensor_scalar(
            counts_m[:], counts_mf[:], float(C - 1), None, op0=ALU.min
        )

        # ---- stage B: one-hot stack [128, C, L] f16 ----
        est = estp.tile([4 * R, C, L], f16)
        for c in range(C):
            nc.vector.tensor_scalar(
                est[:, c, :], counts_m[:], float(c), None, op0=ALU.is_equal
            )

        # ---- stages C/D/E, batched by RB rows for the sqrt table ----
        for rb in range(0, R, RB):
            batch_y2 = []   # (y2 tile, aggr tile, di, r, u)
            veps_all = tinyp.tile([128, 4 * NU], f32, tag="veps")
            for u_r, r in enumerate(range(rb, rb + RB)):
                oh_s = ohp.tile([2 * C, L], f16, tag="oh")
                nc.sync.dma_start(oh_s[0:C, :], est[0 * R + r : 0 * R + r + 1, :, :])
                nc.sync.dma_start(
                    oh_s[C : 2 * C, :], est[1 * R + r : 1 * R + r + 1, :, :]
                )
                oh_d = ohp.tile([2 * C, L], f16, tag="oh")
                nc.sync.dma_start(oh_d[0:C, :], est[2 * R + r : 2 * R + r + 1, :, :])
                nc.sync.dma_start(
                    oh_d[C : 2 * C, :], est[3 * R + r : 3 * R + r + 1, :, :]
                )

                # featT = T2^T @ oh  [F, L]
                featT_s_p = ps_big.tile([F, L], f32, tag="psbig")
                nc.tensor.matmul(featT_s_p[:], t2[:], oh_s[:], start=True, stop=True)
                featT_s = featp.tile([F, L], f16, tag="feat")
                nc.scalar.activation(featT_s[:], featT_s_p[:], AF.Copy)
                featT_d_p = ps_big.tile([F, L], f32, tag="psbig")
                nc.tensor.matmul(featT_d_p[:], t2[:], oh_d[:], start=True, stop=True)
                featT_d = featp.tile([F, L], f16, tag="feat")
                nc.scalar.activation(featT_d[:], featT_d_p[:], AF.Copy)

                # x natural chunks [128, 4, F] (residual + bo), kept in PSUM
                x_s_p = ps_x.tile([128, 4, F], f32, tag="psx")
                x_d_p = ps_x.tile([128, 4, F], f32, tag="psx")
                for c in range(4):
                    nc.tensor.matmul(
                        x_s_p[:, c, :],
                        oh_s[:, c * 128 : (c + 1) * 128],
                        t2x[:],
                        start=True,
                        stop=True,
                        skip_group_check=True,
                    )
                    nc.tensor.matmul(
                        x_d_p[:, c, :],
                        oh_d[:, c * 128 : (c + 1) * 128],
                        t2x[:],
                        start=True,
                        stop=True,
                        skip_group_check=True,
                    )

                for di, (aT, oT, xp) in enumerate(
                    [(featT_s, featT_d, x_s_p), (featT_d, featT_s, x_d_p)]
                ):
                    u = 2 * u_r + di
                    # q path
                    qT_p = ps_big.tile([F, L], f32, tag="psbig")
                    nc.tensor.matmul(qT_p[:], wq[:], aT[:], start=True, stop=True)
                    expQT = workp.tile([F, L], f16, tag="expq")
                    nc.scalar.activation(expQT[:], qT_p[:], AF.Exp)
                    # qsum[l] * sqrt(F) per chunk column (no reciprocal needed:
                    # LN is invariant to positive per-row scaling, so we scale
                    # the residual by qsum instead of scaling attn by 1/qsum)
                    qs_p = ps_tiny.tile([128, 4], f32, tag="pstiny")
                    for c in range(4):
                        nc.tensor.matmul(
                            qs_p[:, c : c + 1],
                            expQT[:, c * 128 : (c + 1) * 128],
                            sqrtf[:],
                            start=True,
                            stop=True,
                            skip_group_check=True,
                        )
                    qsum = tinyp.tile([128, 4], f32, tag="qsum")
                    nc.vector.tensor_copy(qsum[:], qs_p[:])

                    # k path (natural layout)
                    k_p = ps_big.tile([128, 4, F], f32, tag="psbig")
                    for c in range(4):
                        nc.tensor.matmul(
                            k_p[:, c, :],
                            oT[:, c * 128 : (c + 1) * 128],
                            wk[:],
                            start=True,
                            stop=True,
                            skip_group_check=True,
                        )
                    expk = workp.tile([128, 4, F], f16, tag="expk")
                    nc.scalar.activation(expk[:], k_p[:], AF.Exp)

                    # v path
                    v_p = ps_big.tile([128, 4, F], f32, tag="psbig")
                    for c in range(4):
                        nc.tensor.matmul(
                            v_p[:, c, :],
                            oT[:, c * 128 : (c + 1) * 128],
                            wv[:],
                            start=True,
                            stop=True,
                            skip_group_check=True,
                        )
                    vnat = workp.tile([128, 4, F], f16, tag="vnat")
                    nc.vector.tensor_copy(vnat[:], v_p[:])

                    # k-softmax denominator -> per-feature column
                    cs_p = ps_tiny.tile([F, 1], f32, tag="pstiny")
                    for c in range(4):
                        nc.tensor.matmul(
                            cs_p[:],
                            expk[:, c, :],
                            ones128[:],
                            start=(c == 0),
                            stop=(c == 3),
                        )
                    s_col = tinyp.tile([F, 1], f32, tag="scol")
                    nc.vector.reciprocal(s_col[:], cs_p[:])

                    # ctx = diag(1/cs) @ (expk^T @ v)
                    ctx_p = ps_tiny.tile([F, F], f32, tag="pstiny")
                    for c in range(4):
                        nc.tensor.matmul(
                            ctx_p[:],
                            expk[:, c, :],
                            vnat[:, c, :],
                            start=(c == 0),
                            stop=(c == 3),
                        )
                    ctx_sb = tinyp.tile([F, F], f16, tag="ctx")
                    nc.vector.tensor_scalar(
                        ctx_sb[:], ctx_p[:], s_col[:], None, op0=ALU.mult
                    )

                    # attn_preT = ctx^T @ expQT  [F(e), L]
                    ap_p = ps_big.tile([F, L], f32, tag="psbig")
                    nc.tensor.matmul(ap_p[:], ctx_sb[:], expQT[:], start=True, stop=True)
                    apre = workp.tile([F, L], f16, tag="apre")
                    nc.scalar.activation(apre[:], ap_p[:], AF.Copy)

                    # wo_nat = attn_pre @ Wo, copied to SBUF f16 (for Pool add)
                    wo_p = ps_big.tile([128, 4, F], f32, tag="psbig")
                    for c in range(4):
                        nc.tensor.matmul(
                            wo_p[:, c, :],
                            apre[:, c * 128 : (c + 1) * 128],
                            wo[:],
                            start=True,
                            stop=True,
                            skip_group_check=True,
                        )
                    wo_sb = outp.tile([128, 4, F], f16, tag="wosb")
                    nc.scalar.activation(wo_sb[:], wo_p[:], AF.Copy)

                    # xq = x * qsum (LN scale-invariance); y2 = xq + wo (Pool)
                    xq = outp.tile([128, 4, F], f16, tag="xq")
                    for c in range(4):
                        nc.vector.tensor_scalar(
                            xq[:, c, :],
                            xp[:, c, :],
                            qsum[:, c : c + 1],
                            None,
                            op0=ALU.mult,
                        )
                    y2 = outp.tile([128, 4, 65], f32, tag="y2")
                    nc.gpsimd.tensor_tensor(
                        y2[:, :, 0:64], xq[:], wo_sb[:], op=ALU.add
                    )

                    # LN stats (per chunk; HW bn_stats emits exactly 6/partition)
                    stats = tinyp.tile([128, 4, 8], f32, tag="stats")
                    aggr = tinyp.tile([128, 4, 2], f32, tag="aggr")
                    for c in range(4):
                        nc.vector.bn_stats(stats[:, c, 0:6], y2[:, c, 0:64])
                        nc.vector.bn_aggr(aggr[:, c, :], stats[:, c, 0:6])
                    nc.vector.tensor_scalar(
                        veps_all[:, 4 * u : 4 * u + 4],
                        aggr[:, :, 1],
                        EPS,
                        None,
                        op0=ALU.add,
                    )
                    batch_y2.append((y2, aggr, di, r, u))

            # batched sqrt + reciprocal (one ACT table swap per RB rows)
            std_all = tinyp.tile([128, 4 * NU], f32, tag="std")
            nc.scalar.activation(std_all[:], veps_all[:], AF.Sqrt)
            rstd_all = tinyp.tile([128, 4 * NU], f32, tag="rstd")
            nc.vector.reciprocal(rstd_all[:], std_all[:])

            for y2, aggr, di, r, u in batch_y2:
                z = outp.tile([128, 4, F], f32, tag="z")
                for c in range(4):
                    nc.vector.tensor_scalar(
                        z[:, c, :],
                        y2[:, c, 0:64],
                        aggr[:, c, 0:1],
                        rstd_all[:, 4 * u + c : 4 * u + c + 1],
                        op0=ALU.subtract,
                        op1=ALU.mult,
                    )
                zg = outp.tile([128, 4, F], f32, tag="zg")
                nc.gpsimd.tensor_tensor(zg[:], z[:], g4[:], op=ALU.mult)
                fin = outp.tile([128, 4, F], f32, tag="fin")
                nc.gpsimd.tensor_tensor(fin[:], zg[:], b4t[:], op=ALU.add)
                dst = out_d[di, r].rearrange("(c p) f -> p c f", p=128)
                nc.sync.dma_start(dst, fin[:])

    return nc


def _split_multi_waits(nc, maxw=1):
    """This container's walrus accepts at most one sync-wait per TPB
    instruction; hoist extra waits onto NoOps inserted just before."""
    n_split = 0
    for fn in nc.m.functions:
        for bb in fn.blocks:
            new_insts = []
            for ins in bb.instructions:
                si = ins.sync_info
                waits = list(si.on_wait) if si and si.on_wait else []
                if len(waits) > maxw:
                    head, tail = waits[:-maxw], waits[-maxw:]
                    for i in range(0, len(head), maxw):
                        chunk = head[i : i + maxw]
                        nop = mybir.InstNoOp(
                            name=f"{ins.name}_waitsplit{i}",
                            sync_info=mybir.SyncInfo(on_wait=chunk, on_update=[]),
                            bass_nofuse=True,
                            engine=ins.engine,
                        )
                        new_insts.append(nop)
                        n_split += 1
                    si.on_wait = tail
                    ins.sync_info = si
                new_insts.append(ins)
            if len(new_insts) != len(bb.instructions):
                bb.instructions = new_insts
    return n_split


def _get_program(split_waits=False):
    key = "nc_split" if split_waits else "nc"
    if key not in _CACHE:
        nc = _build_program()
        if split_waits:
            _split_multi_waits(nc)
        _CACHE[key] = nc
    return _CACHE[key]


def _install_ntff_hook():
    """Register the axon NTFF profiling hook when the image's antenv lacks
    axon_hooks (profiling-only; grading runs never enter this path)."""
    import types

    try:
        from antenv.axon_hooks import get_axon_ntff_profile_hook  # noqa: F401

        return
    except ImportError:
        pass
    try:
        from trn_agent_boot.trn_boot import _ntff_profile_via_ctypes

        hook = _ntff_profile_via_ctypes("/opt/axon/libaxon_pjrt.so")
    except Exception:
        hook = None
    mod = types.ModuleType("antenv.axon_hooks")
    state = {"hook": hook}
    mod.get_axon_ntff_profile_hook = lambda: state["hook"]
    mod.set_axon_ntff_profile_hook = lambda h: state.update(hook=h)
    import antenv

    sys.modules["antenv.axon_hooks"] = mod
    antenv.axon_hooks = mod

    # avoid remote artifact upload during local profiling
    from concourse import bass_utils as _bu

    _bu.upload_artifacts = lambda tmpdir: tmpdir


def kernel(
    src_ids,
    dst_ids,
    enc_w1,
    enc_b1,
    enc_w2,
    enc_b2,
    Wq,
    Wk,
    Wv,
    Wo,
    bo,
    ln_g,
    ln_b,
):
    global LAST_EXEC_NS, LAST_RESULTS
    src_ids = np.asarray(src_ids).astype(np.int32)
    dst_ids = np.asarray(dst_ids).astype(np.int32)
    enc_w1 = np.asarray(enc_w1, np.float32)
    enc_b1 = np.asarray(enc_b1, np.float32)
    enc_w2 = np.asarray(enc_w2, np.float32)
    enc_b2 = np.asarray(enc_b2, np.float32)
    Wq = np.asarray(Wq, np.float32)
    Wk = np.asarray(Wk, np.float32)
    Wv = np.asarray(Wv, np.float32)
    Wo = np.asarray(Wo, np.float32)
    bo = np.asarray(bo, np.float32)
    ln_g = np.asarray(ln_g, np.float32)
    ln_b = np.asarray(ln_b, np.float32)

    # host precompute: count-encode table T'[c] = relu(c*w1+b1)@w2 + b2
    cvals = np.arange(C, dtype=np.float32)[:, None]  # [C, 1]
    T = np.maximum(cvals @ enc_w1 + enc_b1[None, :], 0.0) @ enc_w2 + enc_b2[None, :]
    t2 = np.ascontiguousarray(np.concatenate([T, T], 0), dtype=np.float16)
    # residual table also carries bo (split across the two summed channels)
    Tx = T + 0.5 * bo[None, :]
    t2x = np.ascontiguousarray(np.concatenate([Tx, Tx], 0), dtype=np.float16)

    g4 = np.ascontiguousarray(np.tile(ln_g[None, :], (128, 4)), np.float32)
    b4t = np.ascontiguousarray(np.tile(ln_b[None, :], (128, 4)), np.float32)
    sqrtf = np.full((F, 1), np.sqrt(F), np.float16)
    ones128 = np.ones((128, 1), np.float16)

    shared = {
        "t2": t2,
        "t2x": t2x,
        "wq": Wq.astype(np.float16),
        "wk": Wk.astype(np.float16),
        "wv": Wv.astype(np.float16),
        "wo": Wo.astype(np.float16),
        "sqrtf": sqrtf,
        "ones128": ones128,
        "g4": g4,
        "b4": b4t,
    }
    in_maps = []
    for core in range(NCORES):
        sl = slice(core * R, (core + 1) * R)
        s, d = src_ids[sl], dst_ids[sl]
        ids_a = np.ascontiguousarray(np.concatenate([s, s, d, d], 0), np.int32)
        ids_b = np.ascontiguousarray(np.concatenate([s, d, d, s], 0), np.int32)
        in_maps.append({"ids_a": ids_a, "ids_b": ids_b, **shared})

    if TRACE:
        _install_ntff_hook()

    nc = _get_program(split_waits=True)
    res = run_bass_kernel_spmd(
        nc, in_maps, list(range(NCORES)), trace=TRACE
    )
    LAST_EXEC_NS = res.exec_time_ns
    LAST_RESULTS = res
    outs = [res.results[i]["out"] for i in range(NCORES)]
    return np.concatenate(outs, axis=1)


if __name__ == "__main__":
    # smoke test with random data
    rng = np.random.default_rng(0)
    ins = {
        "src_ids": rng.integers(0, 2000, (B, L)).astype(np.int32),
        "dst_ids": rng.integers(0, 2000, (B, L)).astype(np.int32),
        "enc_w1": rng.normal(size=(1, F)).astype(np.float32) * 0.05,
        "enc_b1": rng.normal(size=(F,)).astype(np.float32) * 0.05,
        "enc_w2": rng.normal(size=(F, F)).astype(np.float32) * 0.05,
        "enc_b2": rng.normal(size=(F,)).astype(np.float32) * 0.05,
        "Wq": rng.normal(size=(F, F)).astype(np.float32) * 0.05,
        "Wk": rng.normal(size=(F, F)).astype(np.float32) * 0.05,
        "Wv": rng.normal(size=(F, F)).astype(np.float32) * 0.05,
        "Wo": rng.normal(size=(F, F)).astype(np.float32) * 0.05,
        "bo": rng.normal(size=(F,)).astype(np.float32) * 0.05,
        "ln_g": np.ones(F, np.float32),
        "ln_b": np.zeros(F, np.float32),
    }
    out = kernel(**ins)
    print("out", out.shape, out.dtype, float(np.abs(out).max()))
